# revision 28
# baseline (speedup 1.0000x reference)
"""Deep Neural Decision Forest kernel for 8x Trainium2 NeuronCores.

Strategy: data-parallel over batch (4096 -> 8 x 512), batch on the matmul
free (N) dimension throughout, feature dims on partitions. All heavy
matmuls run in fp8 (e4m3) with power-of-2 scales folded into the
host-precomputed weights; contraction pairs are fused into DoubleRow
matmuls (2 k-subtiles per PE cell, 0.5 cycles/row):

  conv1 (Toeplitz DR matmul) -> relu+maxpool (ACT relu + DVE max) ->
  conv2 (Toeplitz DR, ky-pairs incl. a zero ky=5 pad row) -> relu+maxpool
  -> fused per-tree-pair loop: MLP (DR, y-pairs) -> z = w2^T th ->
  e = exp(z), s = softplus(z) = ln(1+e) on ACT (single natural_log_exp
  table, both trees batched into [128,1024] ACT ops) -> merged DR matmul
  per tree computing logmu = w2a^T th - P^T s in one pass (C tile layout
  th|s0|s1|th-copy; GpSimd makes the th copy) -> mu = exp(logmu) fp8 ->
  one DR matmul per tree pair accumulates py += lp^T mu -> ln(py*2^-24).

Scales (all powers of two, exact): x:1, T1:64, H1:64 (= conv1 psum, so
the relu+maxpool chain needs no multiply), T2:1 (conv2 psum 64 = F scale),
W1:64, TH:64, W2:16 (z psum 1024), w2a x2 + P x(-128) (lm psum 128),
MU:128, LP:2^17. Input DMAs are spread over the three DMA-capable engine
queues (sync/scalar/gpsimd) in consumption order.
"""

import numpy as np
import ml_dtypes

import concourse.bass as bass
import concourse.tile as tile
from concourse import bacc, mybir
from concourse.alu_op_type import AluOpType
from concourse.bass_utils import run_bass_kernel_spmd

AF = mybir.ActivationFunctionType
F32 = mybir.dt.float32
BF16 = mybir.dt.bfloat16
FP8 = mybir.dt.float8e4
DR = mybir.MatmulPerfMode.DoubleRow

NDEPTH, NLABEL, NTREE, B = 6, 10, 32, 4096
NLEAF = 128
NCORES = 8
BC = B // NCORES  # 512 batch per core

BF = ml_dtypes.bfloat16
E4 = ml_dtypes.float8_e4m3

USE_SOFTPLUS = False

LN_SMU = float(np.log(128.0))  # mu stored as exp(lm + ln 128)


ENABLE_LDW_OPT = False


def _patch_ldw_opt():
    """Flip walrus --enable-ldw-opt to true: overlaps LDWEIGHTS with the
    previous matmul's streaming (192-256 of ~512 cycles per matmul)."""
    if not ENABLE_LDW_OPT or getattr(bacc, "_ddf_ldw_patch", False):
        return
    from concourse import bass_utils as bu
    orig = bu.run_command

    def patched(argv, **kwargs):
        argv = ["--enable-ldw-opt=true" if a == "--enable-ldw-opt=false" else a
                for a in argv]
        return orig(argv, **kwargs)

    bu.run_command = patched
    bacc._ddf_ldw_patch = True


def _patch_act_tables():
    """Make Exp/Ln resolvable only via natural_log_exp_and_others so the
    table-load inserter cannot ping-pong between the exp-only and ln-only
    sets. Also register Softplus under softplus_and_others (from_pwp maps
    its 'act2' slot to Unknown). Set positions are preserved."""
    if getattr(bacc, "_ddf_act_patch", False):
        return
    import concourse.hw_specs as hs
    orig = hs.get_activation_tables

    def patched(module_arch):
        tabs = orig(module_arch)
        for name, funcs in tabs.items():
            if name != "natural_log_exp_and_others":
                funcs.discard(AF.Exp)
                funcs.discard(AF.Ln)
            if name == "softplus_and_others":
                funcs.add(AF.Softplus)
        return tabs

    bacc.get_activation_tables = patched
    bacc._ddf_act_patch = True


# ---------------------------------------------------------------- host math
def _routing():
    node = np.zeros((NDEPTH + 1, NLEAF), np.int32)
    left = np.zeros((NDEPTH + 1, NLEAF), bool)
    left[0] = np.arange(NLEAF) < NLEAF // 2
    for d in range(1, NDEPTH + 1):
        w = 2 ** (NDEPTH - d + 1)
        j = np.arange(NLEAF)
        node[d] = 2**d - 1 + j // w
        left[d] = (j % w) < w // 2
    return node, left


def _route_mats():
    node, left = _routing()
    A = np.zeros((128, 128), np.float32)
    P = np.zeros((128, 128), np.float32)
    for d in range(NDEPTH + 1):
        for l in range(NLEAF):
            n = node[d, l]
            P[n, l] = 1.0
            if left[d, l]:
                A[n, l] = 1.0
    return A, P


def _conv1_toeplitz(w1c):
    t1 = np.zeros((2, 112, 120), np.float32)
    t2 = np.zeros((2, 112, 120), np.float32)
    for q in range(2):
        for oc in range(10):
            for i in range(12):
                ox = 2 * i + q
                c = oc * 12 + i
                for kx in range(5):
                    px = ox + kx
                    for ky in range(4):
                        t1[q, 28 * ky + px, c] = w1c[oc, 0, ky, kx]
                    t2[q, px, c] = w1c[oc, 0, 4, kx]
    return t1, t2


def _conv2_toeplitz(w2c):
    # [ky, q, 120, 80]: rows r=(ic,px) px 0..11; cols c=(oc,i) ox=2i+q
    t = np.zeros((5, 2, 120, 80), np.float32)
    for ky in range(5):
        for q in range(2):
            for oc in range(20):
                for i in range(4):
                    ox = 2 * i + q
                    c = oc * 4 + i
                    for kx in range(5):
                        px = ox + kx
                        for ic in range(10):
                            t[ky, q, ic * 12 + px, c] = w2c[oc, ic, ky, kx]
    return t


def _q8(x):
    return np.asarray(x, np.float32).astype(E4)


def _precompute(inputs):
    """Host-side derived weights (fp8/bf16 numpy arrays, device layouts)."""
    x = np.asarray(inputs["x"], np.float32).reshape(B, 784)
    w1c = np.asarray(inputs["conv1_w"], np.float32)
    b1c = np.asarray(inputs["conv1_b"], np.float32)
    w2c = np.asarray(inputs["conv2_w"], np.float32)
    b2c = np.asarray(inputs["conv2_b"], np.float32)
    w1 = np.asarray(inputs["w1"], np.float32)   # [T,320,50]
    b1 = np.asarray(inputs["b1"], np.float32)
    w2 = np.asarray(inputs["w2"], np.float32)   # [T,50,128]
    b2 = np.asarray(inputs["b2"], np.float32)
    pi = np.asarray(inputs["pi"], np.float32)   # [T,128,10]

    assert np.all(b1c == 0) and np.all(b2c == 0), "conv biases assumed zero"
    assert np.all(b1 == 0) and np.all(b2 == 0), "mlp biases assumed zero"

    A, P = _route_mats()

    # xtall [112, 4, 7, B]: value x_row(28m+112k+p); rows >= 784 are zero
    xt = np.zeros((896, B), np.float32)
    xt[:784] = x.T
    x8 = _q8(xt).astype(np.float32)  # quantize once, then gather
    xtall = np.zeros((112, 4, 7, B), np.float32)
    for m in range(4):
        for k in range(7):
            xtall[:, m, k, :] = x8[28 * m + 112 * k: 28 * m + 112 * k + 112]
    xtall = xtall.astype(E4)  # exact (already e4m3 values)

    t1, t2 = _conv1_toeplitz(w1c)
    # M padded 120 -> 128: DoubleRow k-subtile stride must be 16-aligned
    tq = np.zeros((112, 2, 2, 128), np.float32)
    for q in range(2):
        tq[:, q, 0, :120] = 64.0 * t1[q]
        tq[:, q, 1, :120] = 64.0 * t2[q]

    w2t5 = _conv2_toeplitz(w2c)
    # M padded 80 -> 96 for the same 16-alignment rule
    w2t = np.zeros((120, 2, 3, 2, 96), np.float32)
    for q in range(2):
        for pair in range(3):
            for sub in range(2):
                ky = 2 * pair + sub
                if ky < 5:
                    w2t[:, q, pair, sub, :80] = w2t5[ky, q]

    # w1p [80, 16, 2, 2, 128]
    w1p = np.zeros((80, 16, 2, 2, 128), np.float32)
    p_idx = np.arange(80)
    oc = p_idx // 4
    i_ = p_idx % 4
    for j in range(16):
        for yp in range(2):
            for sub in range(2):
                y = 2 * yp + sub
                f = oc * 16 + y * 4 + i_  # [80]
                w1p[:, j, yp, sub, 0:50] = 64.0 * w1[2 * j][f, :]
                w1p[:, j, yp, sub, 64:114] = 64.0 * w1[2 * j + 1][f, :]

    # w2z [128, 32*128] fp8; tree t rows (t%2)*64..+50, cols t*128..
    # wm [128, 32, 2, 128] fp8: merged (w2a | -128 P) DoubleRow weights.
    # Even trees read C slots (th, s0) -> subtiles (w2a, -128P); odd trees
    # read (s1, th-copy) -> subtiles swapped. lm psum scale = 128.
    w2q = _q8(16.0 * w2).astype(np.float32) / 16.0  # [T,50,128]
    w2zall = np.zeros((128, 32 * 128), np.float32)
    wm = np.zeros((128, 32, 2, 128), np.float32)
    negp128 = -128.0 * P
    for t in range(32):
        s = t % 2
        w2zall[s * 64:s * 64 + 50, t * 128:(t + 1) * 128] = 16.0 * w2q[t]
        w2a = w2q[t][:, :127] @ A[:127, :]
        w2af = np.zeros((128, 128), np.float32)
        w2af[s * 64:s * 64 + 50, :] = 2.0 * w2a
        if s == 0:
            wm[:, t, 0, :] = w2af
            wm[:, t, 1, :] = negp128
        else:
            wm[:, t, 0, :] = negp128
            wm[:, t, 1, :] = w2af

    pim = pi - pi.max(axis=-1, keepdims=True)
    e = np.exp(pim)
    leafp = e / e.sum(axis=-1, keepdims=True)
    leafp_s = leafp * (2.0 ** 17 / float(NLEAF * NTREE))
    lp = np.zeros((128, 16, 2, 16), np.float32)
    for t in range(32):
        lp[:, t // 2, t % 2, 0:10] = leafp_s[t]

    return dict(
        xtall=xtall,
        tq=tq.astype(E4),
        w2t=w2t.astype(E4),
        w1p=w1p.astype(E4),
        w2z=w2zall.astype(E4),
        wm=wm.astype(E4),
        lp=lp.astype(E4),
    )


# ------------------------------------------------------------- bass program
def _build_nc(n_loop=1):
    _patch_act_tables()
    _patch_ldw_opt()
    nc = bacc.Bacc("TRN2", target_bir_lowering=False, debug=False,
                   num_devices=NCORES)

    # register the mu-bias constant (ln 256) like Bass's built-in consts
    _c = nc.alloc_sbuf_tensor("const-f32-lnsmu", [128, 1], F32)
    nc.gpsimd.memset(_c.ap(), LN_SMU)
    nc.const_aps.aps[(F32, LN_SMU)] = _c.ap()
    nc.all_engine_barrier()

    d_xt = nc.dram_tensor("xtall", [112, 4, 7, BC], FP8,
                          kind="ExternalInput").ap()
    d_tq = nc.dram_tensor("tq", [112, 2, 2, 128], FP8,
                          kind="ExternalInput").ap()
    d_w2t = nc.dram_tensor("w2t", [120, 2, 3, 2, 96], FP8,
                           kind="ExternalInput").ap()
    d_w1p = nc.dram_tensor("w1p", [80, 16, 2, 2, 128], FP8,
                           kind="ExternalInput").ap()
    d_w2z = nc.dram_tensor("w2z", [128, 32 * 128], FP8,
                           kind="ExternalInput").ap()
    d_wm = nc.dram_tensor("wm", [128, 32, 2, 128], FP8,
                          kind="ExternalInput").ap()
    d_lp = nc.dram_tensor("lp", [128, 16, 2, 16], FP8,
                          kind="ExternalInput").ap()
    d_out = nc.dram_tensor("out", [10, BC], F32, kind="ExternalOutput").ap()

    with tile.TileContext(nc) as tc:
        _emit(tc, d_xt, d_tq, d_w2t, d_w1p, d_w2z, d_wm,
              d_lp, d_out, n_loop=n_loop)
    nc.compile()
    return nc


def _emit(tc, d_xt, d_tq, d_w2t, d_w1p, d_w2z, d_wm,
          d_lp, d_out, n_loop=1):
    from contextlib import ExitStack
    nc = tc.nc
    ctx = ExitStack()
    with ctx:
        consts = ctx.enter_context(tc.tile_pool(name="consts", bufs=1))
        work = ctx.enter_context(tc.tile_pool(name="work", bufs=1))
        tmp = ctx.enter_context(tc.tile_pool(name="tmp", bufs=6))
        mup = ctx.enter_context(tc.tile_pool(name="mup", bufs=3))
        ps = ctx.enter_context(tc.tile_pool(name="ps", bufs=3, space="PSUM"))
        pyp = ctx.enter_context(tc.tile_pool(name="pyp", bufs=1, space="PSUM"))

        # ---- load constants, spread across per-engine DMA queues
        # (each DMA_DIRECT2D costs ~670ns issue time on its engine's queue)
        xall = consts.tile([112, 4, 7, BC], FP8, tag="xall")
        xm = {m: xall[:, m, :, :] for m in range(4)}
        tq = consts.tile([112, 2, 2, 128], FP8, tag="tq")
        w2t = consts.tile([120, 2, 3, 2, 96], FP8, tag="w2t")
        w1p = consts.tile([80, 16, 2, 2, 128], FP8, tag="w1p")
        w2z = consts.tile([128, 32 * 128], FP8, tag="w2z")
        wm = consts.tile([128, 32, 2, 128], FP8, tag="wm")
        lp = consts.tile([128, 16, 2, 16], FP8, tag="lp")

        # x tiles split across the 3 DMA-capable queues (~110 GB/s each);
        # m0/m1 gate conv1 r=0, m3 gates r=1
        nc.sync.dma_start(out=tq[:], in_=d_tq)
        nc.sync.dma_start(out=xall[:, 0, :, :], in_=d_xt[:, 0, :, :])
        nc.scalar.dma_start(out=xall[:, 1, :, :], in_=d_xt[:, 1, :, :])
        nc.gpsimd.dma_start(out=xall[:, 2, :, :], in_=d_xt[:, 2, :, :])
        nc.sync.dma_start(out=xall[:, 3, :, :], in_=d_xt[:, 3, :, :])
        nc.gpsimd.dma_start(out=w2t[:], in_=d_w2t)
        nc.scalar.dma_start(out=w1p[:], in_=d_w1p)
        nc.scalar.dma_start(out=w2z[:], in_=d_w2z)
        nc.sync.dma_start(out=wm[:], in_=d_wm)
        nc.gpsimd.dma_start(out=lp[:], in_=d_lp)

        def _compute(it=0):
            # H1 [120, 13, BC] fp8 (block 12 stays zero for the ky=5 pad);
            # F [80, 4, BC] fp8
            h1 = work.tile([120, 13, BC], FP8, tag=f"h1_{it}")
            fy = work.tile([80, 4, BC], FP8, tag=f"fy_{it}")
            nc.gpsimd.memset(h1[:, 12, :], 0.0)

            # ---- conv1 + pool -> H1 blocks r=0..11
            # chain: c_dy = max(psum_q0, psum_q1) on DVE (PSUM-capable),
            # h1 = max(max(c0, 0), c1) on GpSimd (SBUF-only)
            for r in range(12):
                pt = {}
                for dy in range(2):
                    oy = 2 * r + dy
                    m, k = oy % 4, oy // 4
                    p = ps.tile([128, 2, BC], F32, tag="ps")
                    for q in range(2):
                        nc.tensor.matmul(out=p[:, q, :], lhsT=tq[:, q, :, :],
                                         rhs=xm[m][:, k:k + 2, :],
                                         perf_mode=DR, start=True, stop=True)
                    pt[dy] = p
                # pool chain split: ACT relu(q0), DVE max(q1, .), DVE merge
                b = {}
                for dy in range(2):
                    a = tmp.tile([120, BC], BF16, tag="mxa")
                    nc.scalar.activation(out=a[:], in_=pt[dy][:120, 0, :],
                                         func=AF.Relu)
                    bd = tmp.tile([120, BC], BF16, tag="mxb")
                    nc.vector.tensor_max(bd[:], pt[dy][:120, 1, :], a[:])
                    b[dy] = bd
                nc.vector.tensor_max(h1[:, r, :], b[0][:], b[1][:])

            # ---- conv2 + pool -> F blocks y=0..3
            for y in range(4):
                pt = {}
                for dy in range(2):
                    oy = 2 * y + dy
                    p = ps.tile([128, 2, BC], F32, tag="ps")
                    for q in range(2):
                        for pair in range(3):
                            nc.tensor.matmul(
                                out=p[:96, q, :],
                                lhsT=w2t[:, q, pair, :, :],
                                rhs=h1[:, oy + 2 * pair:oy + 2 * pair + 2, :],
                                perf_mode=DR,
                                start=(pair == 0), stop=(pair == 2))
                    pt[dy] = p
                b = {}
                for dy in range(2):
                    a = tmp.tile([80, BC], BF16, tag="mxa2")
                    nc.scalar.activation(out=a[:], in_=pt[dy][:80, 0, :],
                                         func=AF.Relu)
                    bd = tmp.tile([80, BC], BF16, tag="mxb2")
                    nc.vector.tensor_max(bd[:], pt[dy][:80, 1, :], a[:])
                    b[dy] = bd
                nc.vector.tensor_max(fy[:, y, :], b[0][:], b[1][:])

            # ---- fused MLP + B1 per tree pair: th_j (PE+DVE), z (PE),
            # e/s (ACT). Interleaving keeps PE busy under the ACT-bound
            # e/s chain instead of serializing the phases.
            # C_j [128, 4, BC] fp8 = (th | s_t0 | s_t1 | th-copy): even tree
            # reads slots 0:2, odd tree slots 2:4 (wm subtiles swapped), so
            # one DR matmul per tree covers w2a^T th - 128 P^T s.
            cj = {}
            for j in range(16):
                # mlp psum shares the zp tile: slot0 is mlp out, then (after
                # th is extracted) z_t0 overwrites it; z_t1 goes to slot1
                zp = ps.tile([128, 2, BC], F32, tag="ps")
                for yp in range(2):
                    nc.tensor.matmul(out=zp[:, 0, :], lhsT=w1p[:, j, yp, :, :],
                                     rhs=fy[:, 2 * yp:2 * yp + 2, :],
                                     perf_mode=DR,
                                     start=(yp == 0), stop=(yp == 1))
                c = work.tile([128, 4, BC], FP8, tag=f"c_{it}_{j}")
                nc.vector.tensor_scalar(c[:, 0, :], zp[:, 0, :], 2.0 ** -6,
                                        0.0, AluOpType.mult, AluOpType.max)
                nc.gpsimd.tensor_copy(c[:, 3, :], c[:, 0, :])
                cj[j] = c

                for s_ in range(2):
                    t_ = 2 * j + s_
                    r0 = s_ * 64
                    nc.tensor.matmul(out=zp[:, s_, :],
                                     lhsT=w2z[r0:r0 + 50,
                                              t_ * 128:(t_ + 1) * 128],
                                     rhs=c[r0:r0 + 50, 0, :],
                                     start=True, stop=True)
                e = tmp.tile([128, 2, BC], BF16, tag="e")
                nc.scalar.activation(out=e[:], in_=zp[:], func=AF.Exp,
                                     bias=0.0, scale=2.0 ** -10)
                nc.scalar.activation(out=c[:, 1:3, :], in_=e[:], func=AF.Ln,
                                     bias=1.0, scale=1.0)

            # ---- B2: logmu -> mu -> py accumulation
            py = pyp.tile([16, BC], F32, tag="py")
            for j in range(16):
                lm = ps.tile([128, 2, BC], F32, tag="ps")
                for s_ in range(2):
                    t_ = 2 * j + s_
                    nc.tensor.matmul(out=lm[:, s_, :],
                                     lhsT=wm[:, t_, :, :],
                                     rhs=cj[j][:, 2 * s_:2 * s_ + 2, :],
                                     perf_mode=DR, start=True, stop=True)
                mu = mup.tile([128, 2, BC], FP8, tag="mu")
                nc.scalar.activation(out=mu[:], in_=lm[:], func=AF.Exp,
                                     bias=LN_SMU, scale=2.0 ** -7)
                nc.tensor.matmul(out=py[:], lhsT=lp[:, j, :, :], rhs=mu[:],
                                 perf_mode=DR,
                                 start=(j == 0), stop=(j == 15),
                                 skip_group_check=True)

            out_t = work.tile([10, BC], F32, tag=f"out_{it}")
            nc.scalar.activation(out=out_t[:], in_=py[:10, :], func=AF.Ln,
                                 bias=0.0, scale=2.0 ** -24)
            nc.sync.dma_start(out=d_out, in_=out_t[:])

        if n_loop == 1:
            _compute()
        else:
            with tc.For_i(0, n_loop, 1):
                _compute()


_NC_CACHE = None


def _get_nc():
    global _NC_CACHE
    if _NC_CACHE is None:
        _NC_CACHE = _build_nc()
    return _NC_CACHE


def make_in_maps(inputs):
    pre = _precompute(inputs)
    shared = {k: pre[k] for k in
              ("tq", "w2t", "w1p", "w2z", "wm", "lp")}
    in_maps = []
    for c in range(NCORES):
        m = dict(shared)
        m["xtall"] = np.ascontiguousarray(
            pre["xtall"][:, :, :, c * BC:(c + 1) * BC])
        in_maps.append(m)
    return in_maps


def kernel(**inputs):
    nc = _get_nc()
    in_maps = make_in_maps(inputs)
    res = run_bass_kernel_spmd(nc, in_maps, core_ids=list(range(NCORES)))
    outs = [res.results[c]["out"] for c in range(NCORES)]  # each [10, BC]
    full = np.concatenate(outs, axis=1)  # [10, B]
    return np.ascontiguousarray(full.T).astype(np.float32)  # [B, 10]


# revision 30
# speedup vs baseline: 1.0081x; 1.0081x over previous
"""Deep Neural Decision Forest kernel for 8x Trainium2 NeuronCores.

Strategy: data-parallel over batch (4096 -> 8 x 512), batch on the matmul
free (N) dimension throughout, feature dims on partitions. All heavy
matmuls run in fp8 (e4m3) with power-of-2 scales folded into the
host-precomputed weights; contraction pairs are fused into DoubleRow
matmuls (2 k-subtiles per PE cell, 0.5 cycles/row):

  conv1 (Toeplitz DR matmul) -> relu+maxpool (ACT relu + DVE max) ->
  conv2 (Toeplitz DR, ky-pairs incl. a zero ky=5 pad row) -> relu+maxpool
  -> fused per-tree-pair loop: MLP (DR, y-pairs) -> z = w2^T th ->
  e = exp(z), s = softplus(z) = ln(1+e) on ACT (single natural_log_exp
  table, both trees batched into [128,1024] ACT ops) -> merged DR matmul
  per tree computing logmu = w2a^T th - P^T s in one pass (C tile layout
  th|s0|s1|th-copy; GpSimd makes the th copy) -> mu = exp(logmu) fp8 ->
  one DR matmul per tree pair accumulates py += lp^T mu -> ln(py*2^-24).

Scales (all powers of two, exact): x:1, T1:64, H1:64 (= conv1 psum, so
the relu+maxpool chain needs no multiply), T2:1 (conv2 psum 64 = F scale),
W1:64, TH:64, W2:16 (z psum 1024), w2a x2 + P x(-128) (lm psum 128),
MU:128, LP:2^17. Input DMAs are spread over the three DMA-capable engine
queues (sync/scalar/gpsimd) in consumption order.
"""

import numpy as np
import ml_dtypes

import concourse.bass as bass
import concourse.tile as tile
from concourse import bacc, mybir
from concourse.alu_op_type import AluOpType
from concourse.bass_utils import run_bass_kernel_spmd

AF = mybir.ActivationFunctionType
F32 = mybir.dt.float32
BF16 = mybir.dt.bfloat16
FP8 = mybir.dt.float8e4
DR = mybir.MatmulPerfMode.DoubleRow

NDEPTH, NLABEL, NTREE, B = 6, 10, 32, 4096
NLEAF = 128
NCORES = 8
BC = B // NCORES  # 512 batch per core

BF = ml_dtypes.bfloat16
E4 = ml_dtypes.float8_e4m3

USE_SOFTPLUS = False

LN_SMU = float(np.log(128.0))  # mu stored as exp(lm + ln 128)


ENABLE_LDW_OPT = False


def _patch_ldw_opt():
    """Flip walrus --enable-ldw-opt to true: overlaps LDWEIGHTS with the
    previous matmul's streaming (192-256 of ~512 cycles per matmul)."""
    if not ENABLE_LDW_OPT or getattr(bacc, "_ddf_ldw_patch", False):
        return
    from concourse import bass_utils as bu
    orig = bu.run_command

    def patched(argv, **kwargs):
        argv = ["--enable-ldw-opt=true" if a == "--enable-ldw-opt=false" else a
                for a in argv]
        return orig(argv, **kwargs)

    bu.run_command = patched
    bacc._ddf_ldw_patch = True


def _patch_act_tables():
    """Make Exp/Ln resolvable only via natural_log_exp_and_others so the
    table-load inserter cannot ping-pong between the exp-only and ln-only
    sets. Also register Softplus under softplus_and_others (from_pwp maps
    its 'act2' slot to Unknown). Set positions are preserved."""
    if getattr(bacc, "_ddf_act_patch", False):
        return
    import concourse.hw_specs as hs
    orig = hs.get_activation_tables

    def patched(module_arch):
        tabs = orig(module_arch)
        for name, funcs in tabs.items():
            if name != "natural_log_exp_and_others":
                funcs.discard(AF.Exp)
                funcs.discard(AF.Ln)
            if name == "softplus_and_others":
                funcs.add(AF.Softplus)
        return tabs

    bacc.get_activation_tables = patched
    bacc._ddf_act_patch = True


# ---------------------------------------------------------------- host math
def _routing():
    node = np.zeros((NDEPTH + 1, NLEAF), np.int32)
    left = np.zeros((NDEPTH + 1, NLEAF), bool)
    left[0] = np.arange(NLEAF) < NLEAF // 2
    for d in range(1, NDEPTH + 1):
        w = 2 ** (NDEPTH - d + 1)
        j = np.arange(NLEAF)
        node[d] = 2**d - 1 + j // w
        left[d] = (j % w) < w // 2
    return node, left


def _route_mats():
    node, left = _routing()
    A = np.zeros((128, 128), np.float32)
    P = np.zeros((128, 128), np.float32)
    for d in range(NDEPTH + 1):
        for l in range(NLEAF):
            n = node[d, l]
            P[n, l] = 1.0
            if left[d, l]:
                A[n, l] = 1.0
    return A, P


def _conv1_toeplitz(w1c):
    t1 = np.zeros((2, 112, 120), np.float32)
    t2 = np.zeros((2, 112, 120), np.float32)
    for q in range(2):
        for oc in range(10):
            for i in range(12):
                ox = 2 * i + q
                c = oc * 12 + i
                for kx in range(5):
                    px = ox + kx
                    for ky in range(4):
                        t1[q, 28 * ky + px, c] = w1c[oc, 0, ky, kx]
                    t2[q, px, c] = w1c[oc, 0, 4, kx]
    return t1, t2


def _conv2_toeplitz(w2c):
    # [ky, q, 120, 80]: rows r=(ic,px) px 0..11; cols c=(oc,i) ox=2i+q
    t = np.zeros((5, 2, 120, 80), np.float32)
    for ky in range(5):
        for q in range(2):
            for oc in range(20):
                for i in range(4):
                    ox = 2 * i + q
                    c = oc * 4 + i
                    for kx in range(5):
                        px = ox + kx
                        for ic in range(10):
                            t[ky, q, ic * 12 + px, c] = w2c[oc, ic, ky, kx]
    return t


def _q8(x):
    return np.asarray(x, np.float32).astype(E4)


def _precompute(inputs):
    """Host-side derived weights (fp8/bf16 numpy arrays, device layouts)."""
    x = np.asarray(inputs["x"], np.float32).reshape(B, 784)
    w1c = np.asarray(inputs["conv1_w"], np.float32)
    b1c = np.asarray(inputs["conv1_b"], np.float32)
    w2c = np.asarray(inputs["conv2_w"], np.float32)
    b2c = np.asarray(inputs["conv2_b"], np.float32)
    w1 = np.asarray(inputs["w1"], np.float32)   # [T,320,50]
    b1 = np.asarray(inputs["b1"], np.float32)
    w2 = np.asarray(inputs["w2"], np.float32)   # [T,50,128]
    b2 = np.asarray(inputs["b2"], np.float32)
    pi = np.asarray(inputs["pi"], np.float32)   # [T,128,10]

    assert np.all(b1c == 0) and np.all(b2c == 0), "conv biases assumed zero"
    assert np.all(b1 == 0) and np.all(b2 == 0), "mlp biases assumed zero"

    A, P = _route_mats()

    # xtall [112, 4, 7, B]: value x_row(28m+112k+p); rows >= 784 are zero
    xt = np.zeros((896, B), np.float32)
    xt[:784] = x.T
    x8 = _q8(xt).astype(np.float32)  # quantize once, then gather
    xtall = np.zeros((112, 4, 7, B), np.float32)
    for m in range(4):
        for k in range(7):
            xtall[:, m, k, :] = x8[28 * m + 112 * k: 28 * m + 112 * k + 112]
    xtall = xtall.astype(E4)  # exact (already e4m3 values)

    t1, t2 = _conv1_toeplitz(w1c)
    # M padded 120 -> 128: DoubleRow k-subtile stride must be 16-aligned
    tq = np.zeros((112, 2, 2, 128), np.float32)
    for q in range(2):
        tq[:, q, 0, :120] = 64.0 * t1[q]
        tq[:, q, 1, :120] = 64.0 * t2[q]

    w2t5 = _conv2_toeplitz(w2c)
    # M padded 80 -> 96 for the same 16-alignment rule
    w2t = np.zeros((120, 2, 3, 2, 96), np.float32)
    for q in range(2):
        for pair in range(3):
            for sub in range(2):
                ky = 2 * pair + sub
                if ky < 5:
                    w2t[:, q, pair, sub, :80] = w2t5[ky, q]

    # w1p [80, 16, 2, 2, 128]
    w1p = np.zeros((80, 16, 2, 2, 128), np.float32)
    p_idx = np.arange(80)
    oc = p_idx // 4
    i_ = p_idx % 4
    for j in range(16):
        for yp in range(2):
            for sub in range(2):
                y = 2 * yp + sub
                f = oc * 16 + y * 4 + i_  # [80]
                w1p[:, j, yp, sub, 0:50] = 64.0 * w1[2 * j][f, :]
                w1p[:, j, yp, sub, 64:114] = 64.0 * w1[2 * j + 1][f, :]

    # w2z [128, 32*128] fp8; tree t rows (t%2)*64..+50, cols t*128..
    # wm [128, 32, 2, 128] fp8: merged (w2a | -128 P) DoubleRow weights.
    # Even trees read C slots (th, s0) -> subtiles (w2a, -128P); odd trees
    # read (s1, th-copy) -> subtiles swapped. lm psum scale = 128.
    w2q = _q8(16.0 * w2).astype(np.float32) / 16.0  # [T,50,128]
    w2zall = np.zeros((128, 32 * 128), np.float32)
    wm = np.zeros((128, 32, 2, 128), np.float32)
    negp128 = -128.0 * P
    for t in range(32):
        s = t % 2
        w2zall[s * 64:s * 64 + 50, t * 128:(t + 1) * 128] = 16.0 * w2q[t]
        w2a = w2q[t][:, :127] @ A[:127, :]
        w2af = np.zeros((128, 128), np.float32)
        w2af[s * 64:s * 64 + 50, :] = 2.0 * w2a
        if s == 0:
            wm[:, t, 0, :] = w2af
            wm[:, t, 1, :] = negp128
        else:
            wm[:, t, 0, :] = negp128
            wm[:, t, 1, :] = w2af

    pim = pi - pi.max(axis=-1, keepdims=True)
    e = np.exp(pim)
    leafp = e / e.sum(axis=-1, keepdims=True)
    leafp_s = leafp * (2.0 ** 17 / float(NLEAF * NTREE))
    lp = np.zeros((128, 16, 2, 16), np.float32)
    for t in range(32):
        lp[:, t // 2, t % 2, 0:10] = leafp_s[t]

    return dict(
        xtall=xtall,
        tq=tq.astype(E4),
        w2t=w2t.astype(E4),
        w1p=w1p.astype(E4),
        w2z=w2zall.astype(E4),
        wm=wm.astype(E4),
        lp=lp.astype(E4),
    )


# ------------------------------------------------------------- bass program
def _build_nc(n_loop=1):
    _patch_act_tables()
    _patch_ldw_opt()
    nc = bacc.Bacc("TRN2", target_bir_lowering=False, debug=False,
                   num_devices=NCORES)

    # register the mu-bias constant (ln 256) like Bass's built-in consts
    _c = nc.alloc_sbuf_tensor("const-f32-lnsmu", [128, 1], F32)
    nc.gpsimd.memset(_c.ap(), LN_SMU)
    nc.const_aps.aps[(F32, LN_SMU)] = _c.ap()
    nc.all_engine_barrier()

    d_xt = nc.dram_tensor("xtall", [112, 4, 7, BC], FP8,
                          kind="ExternalInput").ap()
    d_tq = nc.dram_tensor("tq", [112, 2, 2, 128], FP8,
                          kind="ExternalInput").ap()
    d_w2t = nc.dram_tensor("w2t", [120, 2, 3, 2, 96], FP8,
                           kind="ExternalInput").ap()
    d_w1p = nc.dram_tensor("w1p", [80, 16, 2, 2, 128], FP8,
                           kind="ExternalInput").ap()
    d_w2z = nc.dram_tensor("w2z", [128, 32 * 128], FP8,
                           kind="ExternalInput").ap()
    d_wm = nc.dram_tensor("wm", [128, 32, 2, 128], FP8,
                          kind="ExternalInput").ap()
    d_lp = nc.dram_tensor("lp", [128, 16, 2, 16], FP8,
                          kind="ExternalInput").ap()
    d_out = nc.dram_tensor("out", [10, BC], F32, kind="ExternalOutput").ap()

    with tile.TileContext(nc) as tc:
        _emit(tc, d_xt, d_tq, d_w2t, d_w1p, d_w2z, d_wm,
              d_lp, d_out, n_loop=n_loop)
    nc.compile()
    return nc


def _emit(tc, d_xt, d_tq, d_w2t, d_w1p, d_w2z, d_wm,
          d_lp, d_out, n_loop=1):
    from contextlib import ExitStack
    nc = tc.nc
    ctx = ExitStack()
    with ctx:
        consts = ctx.enter_context(tc.tile_pool(name="consts", bufs=1))
        work = ctx.enter_context(tc.tile_pool(name="work", bufs=1))
        tmp = ctx.enter_context(tc.tile_pool(name="tmp", bufs=6))
        mup = ctx.enter_context(tc.tile_pool(name="mup", bufs=3))
        ps = ctx.enter_context(tc.tile_pool(name="ps", bufs=3, space="PSUM"))
        pyp = ctx.enter_context(tc.tile_pool(name="pyp", bufs=1, space="PSUM"))

        # ---- load constants, spread across per-engine DMA queues
        # (each DMA_DIRECT2D costs ~670ns issue time on its engine's queue)
        xall = consts.tile([112, 4, 7, BC], FP8, tag="xall")
        xm = {m: xall[:, m, :, :] for m in range(4)}
        tq = consts.tile([112, 2, 2, 128], FP8, tag="tq")
        w2t = consts.tile([120, 2, 3, 2, 96], FP8, tag="w2t")
        w1p = consts.tile([80, 16, 2, 2, 128], FP8, tag="w1p")
        w2z = consts.tile([128, 32 * 128], FP8, tag="w2z")
        wm = consts.tile([128, 32, 2, 128], FP8, tag="wm")
        lp = consts.tile([128, 16, 2, 16], FP8, tag="lp")

        # x tiles split across the 3 DMA-capable queues (~110 GB/s each);
        # m0/m1 gate conv1 r=0, m3 gates r=1
        nc.sync.dma_start(out=tq[:], in_=d_tq)
        nc.sync.dma_start(out=xall[:, 0, :, :], in_=d_xt[:, 0, :, :])
        nc.scalar.dma_start(out=xall[:, 1, :, :], in_=d_xt[:, 1, :, :])
        nc.gpsimd.dma_start(out=xall[:, 2, :, :], in_=d_xt[:, 2, :, :])
        nc.sync.dma_start(out=xall[:, 3, :, :], in_=d_xt[:, 3, :, :])
        nc.gpsimd.dma_start(out=w2t[:], in_=d_w2t)
        nc.scalar.dma_start(out=w1p[:], in_=d_w1p)
        nc.scalar.dma_start(out=w2z[:], in_=d_w2z)
        nc.sync.dma_start(out=wm[:], in_=d_wm)
        nc.gpsimd.dma_start(out=lp[:], in_=d_lp)

        def _compute(it=0):
            # H1 [120, 13, BC] fp8 (block 12 stays zero for the ky=5 pad);
            # F [80, 4, BC] fp8
            h1 = work.tile([120, 13, BC], FP8, tag=f"h1_{it}")
            fy = work.tile([80, 4, BC], FP8, tag=f"fy_{it}")
            nc.gpsimd.memset(h1[:, 12, :], 0.0)

            # ---- conv1 + pool -> H1 blocks r=0..11
            # chain: c_dy = max(psum_q0, psum_q1) on DVE (PSUM-capable),
            # h1 = max(max(c0, 0), c1) on GpSimd (SBUF-only)
            for r in range(12):
                pt = {}
                for dy in range(2):
                    oy = 2 * r + dy
                    m, k = oy % 4, oy // 4
                    p = ps.tile([128, 2, BC], F32, tag="ps")
                    for q in range(2):
                        nc.tensor.matmul(out=p[:, q, :], lhsT=tq[:, q, :, :],
                                         rhs=xm[m][:, k:k + 2, :],
                                         perf_mode=DR, start=True, stop=True)
                    pt[dy] = p
                # pool chain split: ACT relu(q0), DVE max(q1, .), DVE merge
                b = {}
                for dy in range(2):
                    a = tmp.tile([120, BC], BF16, tag="mxa")
                    nc.scalar.activation(out=a[:], in_=pt[dy][:120, 0, :],
                                         func=AF.Relu)
                    bd = tmp.tile([120, BC], BF16, tag="mxb")
                    nc.vector.tensor_max(bd[:], pt[dy][:120, 1, :], a[:])
                    b[dy] = bd
                nc.vector.tensor_max(h1[:, r, :], b[0][:], b[1][:])

            # ---- conv2 + pool -> F blocks y=0..3
            for y in range(4):
                pt = {}
                for dy in range(2):
                    oy = 2 * y + dy
                    p = ps.tile([128, 2, BC], F32, tag="ps")
                    for q in range(2):
                        for pair in range(3):
                            nc.tensor.matmul(
                                out=p[:96, q, :],
                                lhsT=w2t[:, q, pair, :, :],
                                rhs=h1[:, oy + 2 * pair:oy + 2 * pair + 2, :],
                                perf_mode=DR,
                                start=(pair == 0), stop=(pair == 2))
                    pt[dy] = p
                b = {}
                for dy in range(2):
                    a = tmp.tile([80, BC], BF16, tag="mxa2")
                    nc.scalar.activation(out=a[:], in_=pt[dy][:80, 0, :],
                                         func=AF.Relu)
                    bd = tmp.tile([80, BC], BF16, tag="mxb2")
                    nc.vector.tensor_max(bd[:], pt[dy][:80, 1, :], a[:])
                    b[dy] = bd
                nc.vector.tensor_max(fy[:, y, :], b[0][:], b[1][:])

            # ---- fused MLP + B1 per tree pair: th_j (PE+DVE), z (PE),
            # e/s (ACT). Interleaving keeps PE busy under the ACT-bound
            # e/s chain instead of serializing the phases.
            # C_j [128, 4, BC] fp8 = (th | s_t0 | s_t1 | th-copy): even tree
            # reads slots 0:2, odd tree slots 2:4 (wm subtiles swapped), so
            # one DR matmul per tree covers w2a^T th - 128 P^T s.
            cj = {}
            for j in range(16):
                # mlp psum shares the zp tile: slot0 is mlp out, then (after
                # th is extracted) z_t0 overwrites it; z_t1 goes to slot1
                zp = ps.tile([128, 2, BC], F32, tag="ps")
                for yp in range(2):
                    nc.tensor.matmul(out=zp[:, 0, :], lhsT=w1p[:, j, yp, :, :],
                                     rhs=fy[:, 2 * yp:2 * yp + 2, :],
                                     perf_mode=DR,
                                     start=(yp == 0), stop=(yp == 1))
                c = work.tile([128, 4, BC], FP8, tag=f"c_{it}_{j}")
                nc.vector.tensor_scalar(c[:, 0, :], zp[:, 0, :], 2.0 ** -6,
                                        0.0, AluOpType.mult, AluOpType.max)
                nc.gpsimd.tensor_copy(c[:, 3, :], c[:, 0, :])
                cj[j] = c

                for s_ in range(2):
                    t_ = 2 * j + s_
                    r0 = s_ * 64
                    nc.tensor.matmul(out=zp[:, s_, :],
                                     lhsT=w2z[r0:r0 + 50,
                                              t_ * 128:(t_ + 1) * 128],
                                     rhs=c[r0:r0 + 50, 0, :],
                                     start=True, stop=True)
                e = tmp.tile([128, 2, BC], BF16, tag="e")
                nc.scalar.activation(out=e[:], in_=zp[:], func=AF.Exp,
                                     bias=0.0, scale=2.0 ** -10)
                nc.scalar.activation(out=c[:, 1:3, :], in_=e[:], func=AF.Ln,
                                     bias=1.0, scale=1.0)

            # ---- B2: logmu -> mu -> py accumulation
            py = pyp.tile([16, BC], F32, tag="py")
            for j in range(16):
                lm = ps.tile([128, 2, BC], F32, tag="ps")
                for s_ in range(2):
                    t_ = 2 * j + s_
                    nc.tensor.matmul(out=lm[:, s_, :],
                                     lhsT=wm[:, t_, :, :],
                                     rhs=cj[j][:, 2 * s_:2 * s_ + 2, :],
                                     perf_mode=DR, start=True, stop=True)
                mu = mup.tile([128, 2, BC], FP8, tag="mu")
                nc.scalar.activation(out=mu[:], in_=lm[:], func=AF.Exp,
                                     bias=LN_SMU, scale=2.0 ** -7)
                nc.tensor.matmul(out=py[:], lhsT=lp[:, j, :, :], rhs=mu[:],
                                 perf_mode=DR,
                                 start=(j == 0), stop=(j == 15),
                                 skip_group_check=True)

            out_t = work.tile([10, BC], F32, tag=f"out_{it}")
            nc.scalar.activation(out=out_t[:], in_=py[:10, :], func=AF.Ln,
                                 bias=0.0, scale=2.0 ** -24)
            nc.sync.dma_start(out=d_out, in_=out_t[:])

        if n_loop == 1:
            _compute()
        else:
            with tc.For_i(0, n_loop, 1):
                _compute()


_NC_CACHE = None


def _get_nc():
    global _NC_CACHE
    if _NC_CACHE is None:
        _NC_CACHE = _build_nc()
    return _NC_CACHE


def make_in_maps(inputs):
    pre = _precompute(inputs)
    shared = {k: pre[k] for k in
              ("tq", "w2t", "w1p", "w2z", "wm", "lp")}
    in_maps = []
    for c in range(NCORES):
        m = dict(shared)
        m["xtall"] = np.ascontiguousarray(
            pre["xtall"][:, :, :, c * BC:(c + 1) * BC])
        in_maps.append(m)
    return in_maps


def kernel(**inputs):
    nc = _get_nc()
    in_maps = make_in_maps(inputs)
    res = run_bass_kernel_spmd(nc, in_maps, core_ids=list(range(NCORES)))
    outs = [res.results[c]["out"] for c in range(NCORES)]  # each [10, BC]
    full = np.concatenate(outs, axis=1)  # [10, B]
    return np.ascontiguousarray(full.T).astype(np.float32)  # [B, 10]
